# revision 4
# baseline (speedup 1.0000x reference)
"""BaggingMaxPool Trainium2 kernel — LSE matmul + mantissa/exponent-split Ln.

Same structure as kernel2 (S1/S2 via PE matmuls against 0/1 membership),
but beta=23: ln(S) computed via f32 bit-split so the ScalarE Ln table only
ever sees mantissa in [1,2):
    bits = bitcast_u32(S);  k = bits >> 23;  m = (bits & 0x7FFFFF) | 0x3F800000
    ln S = Ln(bitcast_f32(m)) + k*ln2 - 127*ln2
The -127*ln2 offset is folded into downstream constants.
Two-term inversion estimator; epilogue emitted per packed block so it
overlaps the streaming loop.
"""

import numpy as np
import ml_dtypes

import concourse.bass as bass
import concourse.tile as tile
from concourse import bacc, mybir
from concourse.bass_utils import run_bass_kernel_spmd

N = 1024
D = 100000
K = 20
M = 8
DS = D // M          # 12500
FW = 512
NCH = 26             # chunks per core
DP2 = NCH * FW       # 13312
NSLOT = NCH * K      # 520
NB = (NSLOT + 127) // 128   # 5 blocks
BETA = 23.0
CSH = 3.4
LN2 = float(np.log(2.0))
OFF = 127.0 * LN2            # bit-split ln offset
BIAS1 = -BETA * CSH
BIAS2 = -2.0 * BETA * CSH
BIAS3 = -OFF                 # for r = exp(2*l1_off - l2_off - OFF)

F32 = mybir.dt.float32
F16 = mybir.dt.float16
BF16 = mybir.dt.bfloat16
U32 = mybir.dt.uint32
AF = mybir.ActivationFunctionType
ALU = mybir.AluOpType


def build_kernel(dbg: bool = False):
    nc = bacc.Bacc("TRN2", target_bir_lowering=False, debug=False, num_devices=M)
    for val in (BIAS1, BIAS2, BIAS3):
        t = nc.alloc_sbuf_tensor(f"const-f32-{val}", [128, 1], F32)
        nc.gpsimd.memset(t.ap(), val)
        nc.const_aps.aps[(F32, val)] = t.ap()
    nc.all_engine_barrier()
    inp = nc.dram_tensor("inp", [N, DP2], F32, kind="ExternalInput")
    mmat_d = nc.dram_tensor("mmat", [128, 8 * K], BF16, kind="ExternalInput")
    sel_d = nc.dram_tensor("sel", [128, NB * NCH], BF16, kind="ExternalInput")
    out = nc.dram_tensor("out", [NCH, FW], F32, kind="ExternalOutput")
    if dbg:
        out_rs = nc.dram_tensor("out_rs", [128, NB * 2 * FW], F32,
                                kind="ExternalOutput")
        out_mx = nc.dram_tensor("out_mx", [128, NB * FW], F32,
                                kind="ExternalOutput")

    with tile.TileContext(nc) as tc:
        with (
            tc.tile_pool(name="xpool", bufs=2) as xpool,
            tc.tile_pool(name="epool", bufs=2) as epool,
            tc.tile_pool(name="lpool", bufs=2) as lpool,
            tc.tile_pool(name="bpool", bufs=2) as bpool,
            tc.tile_pool(name="ppool", bufs=2, space="PSUM") as ppool,
            tc.tile_pool(name="rpool", bufs=1) as rpool,
        ):
            mm = rpool.tile([128, 8, K], BF16)
            nc.sync.dma_start(mm[:], mmat_d.ap().rearrange("p (w k) -> p w k", w=8))
            sel = rpool.tile([128, NB, NCH], BF16)
            nc.sync.dma_start(sel[:], sel_d.ap().rearrange("p (b c) -> p b c", b=NB))

            # packed raw S1|S2 per (chunk, round) slot: slot j = c*K + k
            rs = rpool.tile([128, NB, 2 * FW], F32)
            nc.vector.memset(rs[:], 1.0)
            mxf = rpool.tile([128, NB, FW], F16)

            inp_r = inp.ap().rearrange("(r p) d -> p r d", p=128)

            def epilogue_block(b):
                """Two-term inversion on packed block b -> mxf[:, b, :]."""
                bits = rs[:, b, :].bitcast(U32)            # [128, 2*FW]
                ki = bpool.tile([128, 2 * FW], U32, name=f"ki{b}", tag="ki")
                nc.vector.tensor_scalar(
                    ki[:], bits, 23, None, ALU.logical_shift_right
                )
                kf = bpool.tile([128, 2 * FW], F32, name=f"kf{b}", tag="kf")
                nc.vector.tensor_copy(kf[:], ki[:])
                mi = bpool.tile([128, 2 * FW], U32, name=f"mi{b}", tag="mi")
                nc.vector.tensor_scalar(
                    mi[:], bits, 0x007FFFFF, 0x3F800000,
                    ALU.bitwise_and, ALU.bitwise_or,
                )
                lm = bpool.tile([128, 2 * FW], F32, name=f"lm{b}", tag="lm")
                nc.scalar.activation(lm[:], mi[:].bitcast(F32), AF.Ln)
                # l_off = ln(S) + 127*ln2 = kf*ln2 + ln(m)
                lo = bpool.tile([128, 2 * FW], F32, name=f"lo{b}", tag="lo")
                nc.vector.scalar_tensor_tensor(
                    lo[:], kf[:], LN2, lm[:], ALU.mult, ALU.add
                )
                l1v = lo[:, 0:FW]
                l2v = lo[:, FW:2 * FW]
                zt = bpool.tile([128, FW], F32, name=f"zt{b}", tag="zt")
                nc.vector.scalar_tensor_tensor(
                    zt[:], l1v, 2.0, l2v, ALU.mult, ALU.subtract
                )
                # r = exp(z - OFF);  q = min(r-1, 1)
                rt = bpool.tile([128, FW], F32, name=f"rt{b}", tag="rt")
                nc.scalar.activation(rt[:], zt[:], AF.Exp, bias=BIAS3)
                qt = bpool.tile([128, FW], F32, name=f"qt{b}", tag="qt")
                nc.vector.tensor_scalar(
                    qt[:], rt[:], 1.0, 1.0, ALU.subtract, ALU.min
                )
                # s = 1 - q^2 ; root = sqrt(s) ; g = ln(1 + root)
                st2 = bpool.tile([128, FW], F32, name=f"st{b}", tag="st2")
                nc.scalar.activation(st2[:], qt[:], AF.Square)
                nc.vector.tensor_scalar(
                    st2[:], st2[:], -1.0, 1.0, ALU.mult, ALU.add
                )
                nc.scalar.activation(st2[:], st2[:], AF.Sqrt)
                nc.scalar.activation(st2[:], st2[:], AF.Ln, bias=1.0)
                # h = (g + (2*b*c - ln2 - OFF)) + l2_off ; mx = h/(2b)
                nc.vector.scalar_tensor_tensor(
                    zt[:], st2[:], 2.0 * BETA * CSH - LN2 - OFF, l2v,
                    ALU.add, ALU.add,
                )
                nc.vector.tensor_scalar(
                    mxf[:, b, :], zt[:], 1.0 / (2.0 * BETA), None, ALU.mult
                )

            done_b = 0
            for c in range(NCH):
                f0 = c * FW
                xt = xpool.tile([128, 8, FW], F32, name=f"xt{c}", tag="xt")
                nc.sync.dma_start(xt[:], inp_r[:, :, f0:f0 + FW])
                xp = epool.tile([128, 8, 2 * FW], BF16, name=f"xp{c}", tag="xp")
                nc.scalar.activation(
                    xp[:, :, 0:FW], xt[:], AF.Exp,
                    bias=BIAS1, scale=BETA,
                )
                nc.vector.tensor_mul(
                    xp[:, :, FW:2 * FW], xp[:, :, 0:FW], xp[:, :, 0:FW]
                )
                ps = ppool.tile([K, 2, FW], F32, name=f"ps{c}", tag="ps")
                for w in range(8):
                    for j in range(2):
                        nc.tensor.matmul(
                            ps[:, j, :],
                            mm[:, w, :],
                            xp[:, w, j * FW:(j + 1) * FW],
                            start=(w == 0),
                            stop=(w == 7),
                        )
                st = lpool.tile([K, 2 * FW], F32, name=f"st{c}", tag="st")
                nc.scalar.activation(st[:], ps[:], AF.Copy)
                # repack raw S into rs slots c*K .. c*K+19
                j0 = c * K
                p0 = j0 % 128
                b0 = j0 // 128
                n1 = min(K, 128 - p0)
                nc.sync.dma_start(rs[p0:p0 + n1, b0, :], st[0:n1, :])
                if n1 < K:
                    nc.sync.dma_start(
                        rs[0:K - n1, b0 + 1, :], st[n1:K, :]
                    )
                # emit epilogue for any block fully repacked
                while done_b < NB and (c + 1) * K >= 128 * (done_b + 1):
                    epilogue_block(done_b)
                    done_b += 1

            while done_b < NB:
                epilogue_block(done_b)
                done_b += 1

            if dbg:
                nc.sync.dma_start(
                    out_rs.ap().rearrange("p (b f) -> p b f", b=NB), rs[:]
                )
                mxd = rpool.tile([128, NB, FW], F32)
                nc.vector.tensor_copy(mxd[:], mxf[:])
                nc.sync.dma_start(
                    out_mx.ap().rearrange("p (b f) -> p b f", b=NB), mxd[:]
                )

            # mean over k: per block matmul with 0/1 chunk-selector, accumulate
            pm = ppool.tile([NCH, FW], F32, name="pm", tag="pm")
            for b in range(NB):
                nc.tensor.matmul(
                    pm[:],
                    sel[:, b, :],
                    mxf[:, b, :],
                    start=(b == 0),
                    stop=(b == NB - 1),
                )
            outs = rpool.tile([NCH, FW], F32)
            nc.scalar.activation(outs[:], pm[:], AF.Copy, scale=1.0 / K)
            nc.sync.dma_start(out.ap(), outs[:])

    nc.compile()
    return nc


def prep_inputs(inp: np.ndarray, indices: np.ndarray):
    inp = np.ascontiguousarray(inp, dtype=np.float32)
    indices = np.asarray(indices)

    mmat = np.zeros((128, 8, K), dtype=ml_dtypes.bfloat16)
    for k in range(K):
        for n in np.unique(indices[k].astype(np.int64)):
            mmat[n % 128, n // 128, k] = 1.0
    mmat = mmat.reshape(128, 8 * K)

    sel = np.zeros((128, NB, NCH), dtype=ml_dtypes.bfloat16)
    for c in range(NCH):
        for k in range(K):
            j = c * K + k
            sel[j % 128, j // 128, c] = 1.0
    sel = sel.reshape(128, NB * NCH)

    in_maps = []
    for c in range(M):
        shard = inp[:, c * DS:(c + 1) * DS]
        shard = np.pad(shard, ((0, 0), (0, DP2 - DS)), mode="edge")
        in_maps.append({
            "inp": np.ascontiguousarray(shard),
            "mmat": mmat,
            "sel": sel,
        })
    return in_maps


def assemble_output(results) -> np.ndarray:
    parts = []
    for c in range(M):
        r = np.asarray(results[c]["out"])
        parts.append(r.reshape(-1)[:DS])
    return np.concatenate(parts)[None, :].astype(np.float32)


_NC_CACHE = {}


def kernel(inp: np.ndarray, indices: np.ndarray) -> np.ndarray:
    if "nc" not in _NC_CACHE:
        _NC_CACHE["nc"] = build_kernel()
    nc = _NC_CACHE["nc"]
    in_maps = prep_inputs(inp, indices)
    res = run_bass_kernel_spmd(nc, in_maps, core_ids=list(range(M)))
    return assemble_output(res.results)


# revision 5
# speedup vs baseline: 1.1467x; 1.1467x over previous
"""BaggingMaxPool Trainium2 kernel — LSE matmul + mantissa/exponent-split Ln.

Same structure as kernel2 (S1/S2 via PE matmuls against 0/1 membership),
but beta=23: ln(S) computed via f32 bit-split so the ScalarE Ln table only
ever sees mantissa in [1,2):
    bits = bitcast_u32(S);  k = bits >> 23;  m = (bits & 0x7FFFFF) | 0x3F800000
    ln S = Ln(bitcast_f32(m)) + k*ln2 - 127*ln2
The -127*ln2 offset is folded into downstream constants.
Two-term inversion estimator; epilogue emitted per packed block so it
overlaps the streaming loop.
"""

import numpy as np
import ml_dtypes

import concourse.bass as bass
import concourse.tile as tile
from concourse import bacc, mybir
from concourse.bass_utils import run_bass_kernel_spmd

N = 1024
D = 100000
K = 20
M = 8
DS = D // M          # 12500
FW = 512
NCH = 26             # chunks per core
DP2 = NCH * FW       # 13312
NSLOT = NCH * K      # 520
NB = (NSLOT + 127) // 128   # 5 blocks
BETA = 23.0
CSH = 3.4
LN2 = float(np.log(2.0))
OFF = 127.0 * LN2            # bit-split ln offset
BIAS1 = -BETA * CSH
BIAS2 = -2.0 * BETA * CSH
BIAS3 = -OFF                 # for r = exp(2*l1_off - l2_off - OFF)

F32 = mybir.dt.float32
F16 = mybir.dt.float16
BF16 = mybir.dt.bfloat16
U32 = mybir.dt.uint32
AF = mybir.ActivationFunctionType
ALU = mybir.AluOpType


def build_kernel(dbg: bool = False):
    nc = bacc.Bacc("TRN2", target_bir_lowering=False, debug=False, num_devices=M)
    for val in (BIAS1, BIAS2, BIAS3):
        t = nc.alloc_sbuf_tensor(f"const-f32-{val}", [128, 1], F32)
        nc.gpsimd.memset(t.ap(), val)
        nc.const_aps.aps[(F32, val)] = t.ap()
    nc.all_engine_barrier()
    inp = nc.dram_tensor("inp", [N, DP2], F32, kind="ExternalInput")
    mmat_d = nc.dram_tensor("mmat", [128, 8 * K], BF16, kind="ExternalInput")
    sel_d = nc.dram_tensor("sel", [128, NB * NCH], BF16, kind="ExternalInput")
    out = nc.dram_tensor("out", [NCH, FW], F32, kind="ExternalOutput")
    if dbg:
        out_rs = nc.dram_tensor("out_rs", [128, NB * 2 * FW], F32,
                                kind="ExternalOutput")
        out_mx = nc.dram_tensor("out_mx", [128, NB * FW], F32,
                                kind="ExternalOutput")

    with tile.TileContext(nc) as tc:
        with (
            tc.tile_pool(name="xpool", bufs=3) as xpool,
            tc.tile_pool(name="epool", bufs=4) as epool,
            tc.tile_pool(name="lpool", bufs=2) as lpool,
            tc.tile_pool(name="bpool", bufs=1) as bpool,
            tc.tile_pool(name="ppool", bufs=2, space="PSUM") as ppool,
            tc.tile_pool(name="rpool", bufs=1) as rpool,
        ):
            mm = rpool.tile([128, 8, K], BF16)
            nc.sync.dma_start(mm[:], mmat_d.ap().rearrange("p (w k) -> p w k", w=8))
            sel = rpool.tile([128, NB, NCH], BF16)
            nc.sync.dma_start(sel[:], sel_d.ap().rearrange("p (b c) -> p b c", b=NB))

            # packed raw S1|S2 per (chunk, round) slot: slot j = c*K + k
            rs = rpool.tile([128, NB, 2 * FW], F32)
            nc.vector.memset(rs[:], 1.0)
            mxf = rpool.tile([128, NB, FW], F16)

            inp_r = inp.ap().rearrange("(r p) d -> p r d", p=128)

            def epilogue_block(b):
                """Two-term inversion on packed block b -> mxf[:, b, :]."""
                bits = rs[:, b, :].bitcast(U32)            # [128, 2*FW]
                ki = bpool.tile([128, 2 * FW], U32, name=f"ki{b}", tag="ki")
                nc.vector.tensor_scalar(
                    ki[:], bits, 23, None, ALU.logical_shift_right
                )
                kf = bpool.tile([128, 2 * FW], F32, name=f"kf{b}", tag="kf")
                nc.vector.tensor_copy(kf[:], ki[:])
                mi = bpool.tile([128, 2 * FW], U32, name=f"mi{b}", tag="mi")
                nc.vector.tensor_scalar(
                    mi[:], bits, 0x007FFFFF, 0x3F800000,
                    ALU.bitwise_and, ALU.bitwise_or,
                )
                lm = bpool.tile([128, 2 * FW], F32, name=f"lm{b}", tag="lm")
                nc.scalar.activation(lm[:], mi[:].bitcast(F32), AF.Ln)
                # l_off = ln(S) + 127*ln2 = kf*ln2 + ln(m)
                lo = bpool.tile([128, 2 * FW], F32, name=f"lo{b}", tag="lo")
                nc.vector.scalar_tensor_tensor(
                    lo[:], kf[:], LN2, lm[:], ALU.mult, ALU.add
                )
                l1v = lo[:, 0:FW]
                l2v = lo[:, FW:2 * FW]
                zt = bpool.tile([128, FW], F32, name=f"zt{b}", tag="zt")
                nc.vector.scalar_tensor_tensor(
                    zt[:], l1v, 2.0, l2v, ALU.mult, ALU.subtract
                )
                # r = exp(z - OFF);  q = min(r-1, 1)
                rt = bpool.tile([128, FW], F32, name=f"rt{b}", tag="rt")
                nc.scalar.activation(rt[:], zt[:], AF.Exp, bias=BIAS3)
                qt = bpool.tile([128, FW], F32, name=f"qt{b}", tag="qt")
                nc.vector.tensor_scalar(
                    qt[:], rt[:], 1.0, 1.0, ALU.subtract, ALU.min
                )
                # s = 1 - q^2 ; root = sqrt(s) ; g = ln(1 + root)
                st2 = bpool.tile([128, FW], F32, name=f"st{b}", tag="st2")
                nc.vector.tensor_mul(st2[:], qt[:], qt[:])
                nc.vector.tensor_scalar(
                    st2[:], st2[:], -1.0, 1.0, ALU.mult, ALU.add
                )
                nc.scalar.activation(st2[:], st2[:], AF.Sqrt)
                nc.scalar.activation(st2[:], st2[:], AF.Ln, bias=1.0)
                # h = (g + (2*b*c - ln2 - OFF)) + l2_off ; mx = h/(2b)
                nc.vector.scalar_tensor_tensor(
                    zt[:], st2[:], 2.0 * BETA * CSH - LN2 - OFF, l2v,
                    ALU.add, ALU.add,
                )
                nc.vector.tensor_scalar(
                    mxf[:, b, :], zt[:], 1.0 / (2.0 * BETA), None, ALU.mult
                )

            def emit_front(c):
                """DMA + exp + square for chunk c; returns xp tile."""
                f0 = c * FW
                xt = xpool.tile([128, 8, FW], F32, name=f"xt{c}", tag="xt")
                nc.sync.dma_start(xt[:], inp_r[:, :, f0:f0 + FW])
                xp = epool.tile([128, 8, 2 * FW], BF16, name=f"xp{c}", tag="xp")
                nc.scalar.activation(
                    xp[:, :, 0:FW], xt[:], AF.Exp,
                    bias=BIAS1, scale=BETA,
                )
                nc.vector.tensor_mul(
                    xp[:, :, FW:2 * FW], xp[:, :, 0:FW], xp[:, :, 0:FW]
                )
                return xp

            def emit_back(c, xp):
                """Matmuls + psum drain + repack for chunk c."""
                ps = ppool.tile([K, 2, FW], F32, name=f"ps{c}", tag="ps")
                for w in range(8):
                    for j in range(2):
                        nc.tensor.matmul(
                            ps[:, j, :],
                            mm[:, w, :],
                            xp[:, w, j * FW:(j + 1) * FW],
                            start=(w == 0),
                            stop=(w == 7),
                        )
                st = lpool.tile([K, 2 * FW], F32, name=f"st{c}", tag="st")
                nc.scalar.activation(st[:], ps[:], AF.Copy)
                j0 = c * K
                p0 = j0 % 128
                b0 = j0 // 128
                n1 = min(K, 128 - p0)
                nc.sync.dma_start(rs[p0:p0 + n1, b0, :], st[0:n1, :])
                if n1 < K:
                    nc.sync.dma_start(
                        rs[0:K - n1, b0 + 1, :], st[n1:K, :]
                    )

            done_b = 0
            for cp in range(0, NCH, 2):
                cs = [c for c in (cp, cp + 1) if c < NCH]
                xps = [emit_front(c) for c in cs]
                for c, xp in zip(cs, xps):
                    emit_back(c, xp)
                # emit epilogue for any block fully repacked
                c_last = cs[-1]
                while done_b < NB and (c_last + 1) * K >= 128 * (done_b + 1):
                    epilogue_block(done_b)
                    done_b += 1

            while done_b < NB:
                epilogue_block(done_b)
                done_b += 1

            if dbg:
                nc.sync.dma_start(
                    out_rs.ap().rearrange("p (b f) -> p b f", b=NB), rs[:]
                )
                mxd = rpool.tile([128, NB, FW], F32)
                nc.vector.tensor_copy(mxd[:], mxf[:])
                nc.sync.dma_start(
                    out_mx.ap().rearrange("p (b f) -> p b f", b=NB), mxd[:]
                )

            # mean over k: per block matmul with 0/1 chunk-selector, accumulate
            pm = ppool.tile([NCH, FW], F32, name="pm", tag="pm")
            for b in range(NB):
                nc.tensor.matmul(
                    pm[:],
                    sel[:, b, :],
                    mxf[:, b, :],
                    start=(b == 0),
                    stop=(b == NB - 1),
                )
            outs = rpool.tile([NCH, FW], F32)
            nc.scalar.activation(outs[:], pm[:], AF.Copy, scale=1.0 / K)
            nc.sync.dma_start(out.ap(), outs[:])

    nc.compile()
    return nc


def prep_inputs(inp: np.ndarray, indices: np.ndarray):
    inp = np.ascontiguousarray(inp, dtype=np.float32)
    indices = np.asarray(indices)

    mmat = np.zeros((128, 8, K), dtype=ml_dtypes.bfloat16)
    for k in range(K):
        for n in np.unique(indices[k].astype(np.int64)):
            mmat[n % 128, n // 128, k] = 1.0
    mmat = mmat.reshape(128, 8 * K)

    sel = np.zeros((128, NB, NCH), dtype=ml_dtypes.bfloat16)
    for c in range(NCH):
        for k in range(K):
            j = c * K + k
            sel[j % 128, j // 128, c] = 1.0
    sel = sel.reshape(128, NB * NCH)

    in_maps = []
    for c in range(M):
        shard = inp[:, c * DS:(c + 1) * DS]
        shard = np.pad(shard, ((0, 0), (0, DP2 - DS)), mode="edge")
        in_maps.append({
            "inp": np.ascontiguousarray(shard),
            "mmat": mmat,
            "sel": sel,
        })
    return in_maps


def assemble_output(results) -> np.ndarray:
    parts = []
    for c in range(M):
        r = np.asarray(results[c]["out"])
        parts.append(r.reshape(-1)[:DS])
    return np.concatenate(parts)[None, :].astype(np.float32)


_NC_CACHE = {}


def kernel(inp: np.ndarray, indices: np.ndarray) -> np.ndarray:
    if "nc" not in _NC_CACHE:
        _NC_CACHE["nc"] = build_kernel()
    nc = _NC_CACHE["nc"]
    in_maps = prep_inputs(inp, indices)
    res = run_bass_kernel_spmd(nc, in_maps, core_ids=list(range(M)))
    return assemble_output(res.results)
